# revision 11
# baseline (speedup 1.0000x reference)
"""Fused LayerNorm + causal multi-head attention (with additive bias) + out-proj
for Trainium2, SPMD over 8 NeuronCores.

Sharding: tensor-parallel over heads. 16 heads / 8 cores = 2 heads per core.
Each core computes LN(x) (replicated), the qkv projection restricted to its
2 heads' columns, causal softmax attention with its heads' bias slices, and a
partial output projection (its heads' rows of w_out). Host sums the 8 partial
outputs (the TP all-reduce, done on gather) in f32 from bf16 partials.

Key layout/algorithm choices (v2, tuned from the v1 trace):
 - x arrives pre-transposed ([dim, token], bf16). The LN mean-subtraction is
   folded into the weights on the HOST: W'' = gammaW - (1/D) 1 colsum(gammaW),
   so x^T W'' = (x - mu)^T gammaW directly. Only rsig (1/std) is applied on
   device, riding the PSUM->SBUF eviction (DVE multiply against a broadcast
   rsig row). The beta term is a rank-1 PSUM matmul (skipped when beta == 0).
 - LN variance stats: slab pre-sums of x and x^2 run on the DVE as two
   strided tensor-tensor adds per level (one instruction does 4 adds), the
   last level on GpSimd; the 128-partition reduction is a single ones-vector
   matmul per 512-token block (PE cost 8k cycles vs 65k for direct matmuls).
   x^2 is produced by one ScalarE Square over [128, 8, 512] per block.
 - Scores are computed transposed, S^T[j, i] = (k_j . q_i); the two heads'
   C=64 matmuls auto-pair into PE row-groups (0,0)/(64,0) and run
   concurrently.
 - The causal mask and softmax max-subtraction are folded into the host bias
   (pre-masked with -1e9; logits are O(10) so exp never overflows).
 - The bias add runs on the PE as TWO 64x64-quadrant identity matmuls
   (quadrants (0,0) and (64,64)) which also run concurrently - half the PE
   cost of a full 128-contraction identity accumulate.
 - exp() instructions are the attention bottleneck (~293ns fixed overhead
   each); score PSUM tiles are [128, 2, 512] spanning two banks so one
   ACTIVATE covers two j-tiles. The first diagonal j-tile is computed
   full-width (its masked region gets -1e9 from the bias, exp -> 0) so it can
   pair with its predecessor; the remaining two diagonal tiles are trimmed
   singles.
 - Softmax normalization is deferred: an all-ones column appended to V gives
   the row sums l_i for free; 1/l is applied to O^T after P@V.
 - The out-projection is interleaved per 512-token i-tile right after its
   attention completes, sharing the score PSUM pool, so the y writeback DMA
   (bf16) overlaps attention compute instead of trailing the kernel.
"""

import numpy as np
import ml_dtypes
from contextlib import ExitStack

import concourse.bass as bass
import concourse.tile as tile
from concourse import bacc, mybir
from concourse.bass_utils import run_bass_kernel_spmd

F32 = mybir.dt.float32
BF16 = mybir.dt.bfloat16
AL = mybir.AluOpType
AF = mybir.ActivationFunctionType

N_CORES = 8
B = 2            # batch
N = 2048         # tokens
D = 1024         # model dim
H = 16           # total heads
HL = 2           # heads per core
DH = 64          # head dim
COLS = 3 * HL * DH   # 384 qkv columns per core
KS = D // 128    # 8 contraction slabs
TT = N // 128    # 16 token tiles
IT = N // 512    # 4 i-tiles (query tiles of 512)
SCALE = DH ** -0.5
LN_EPS = 1e-5
NEG = -1.0e9


def _attn_groups(t, pair_exp=True):
    """j-tile exp groups for i-tile t: list of lists of (j, off, pv_off).

    off: left trim applied to score/bias/exp (0 = full width).
    pv_off: left trim for the P@V output columns (causally-zero region).
    """
    groups = []
    if not pair_exp:
        for j in range(4 * (t + 1)):
            off = max(0, 128 * j - 512 * t)
            groups.append([(j, off, off)])
        return groups
    for k in range(2 * t):
        groups.append([(2 * k, 0, 0), (2 * k + 1, 0, 0)])
    # diagonal-adjacent pair: second tile computed full width (bias -1e9
    # covers its masked region, exp -> 0) so the pair shares one ACTIVATE
    groups.append([(4 * t, 0, 0), (4 * t + 1, 0, 128)])
    groups.append([(4 * t + 2, 256, 256)])
    groups.append([(4 * t + 3, 384, 384)])
    return groups


def build_program(bw_zero=True, debug=False, quad_bias=True, pair_exp=True):
    nc = bacc.Bacc("TRN2", target_bir_lowering=False, debug=False)

    xT_in = nc.dram_tensor("xT", [B, D, N], BF16, kind="ExternalInput")
    biasT_in = nc.dram_tensor("biasT", [HL, N, N], BF16, kind="ExternalInput")
    wqkv_in = nc.dram_tensor("wqkv", [D, COLS], BF16, kind="ExternalInput")
    wout_in = nc.dram_tensor("wout", [HL * DH, D], BF16, kind="ExternalInput")
    bw_in = nc.dram_tensor("bw", [1, COLS], BF16, kind="ExternalInput")
    ident_in = nc.dram_tensor("ident", [128, 128], BF16, kind="ExternalInput")
    y_out = nc.dram_tensor("y", [B, N, D], BF16, kind="ExternalOutput")
    if debug:
        dq_out = nc.dram_tensor("dq", [B, 3, 128, N], BF16, kind="ExternalOutput")
        drs_out = nc.dram_tensor("drs", [B, 128, N], BF16, kind="ExternalOutput")
        do_out = nc.dram_tensor("do", [B, 128, N], BF16, kind="ExternalOutput")
        dva_out = nc.dram_tensor("dva", [B, 128, TT * 130], BF16, kind="ExternalOutput")
        drec_out = nc.dram_tensor("drec", [B, HL, IT, 512], BF16, kind="ExternalOutput")

    MM = dict(skip_group_check=True)

    with tile.TileContext(nc) as tc, ExitStack() as ctx:
        # ---- persistent sbuf ----
        pers = ctx.enter_context(tc.tile_pool(name="pers", bufs=1))
        qT = [pers.tile([128, N], BF16, tag=f"qT{b}", name=f"qT{b}") for b in range(B)]
        kT = [pers.tile([128, N], BF16, tag=f"kT{b}", name=f"kT{b}") for b in range(B)]
        vT = [pers.tile([128, N], BF16, tag=f"vT{b}", name=f"vT{b}") for b in range(B)]
        # V natural with ones column: per key-tile [.., 130]: h0 v(64)+1, h1 v(64)+1
        vA = [pers.tile([128, TT, 130], BF16, tag=f"vA{b}", name=f"vA{b}") for b in range(B)]
        oT = [pers.tile([128, N], BF16, tag=f"oT{b}", name=f"oT{b}") for b in range(B)]
        ident = pers.tile([128, 128], BF16, tag="ident")
        nc.sync.dma_start(ident[:], ident_in.ap())
        wqb = pers.tile([128, KS, COLS], BF16, tag="wqb")
        nc.sync.dma_start(wqb[:], wqkv_in.ap().rearrange("(k p) c -> p k c", p=128))
        wob = pers.tile([128, D], BF16, tag="wob")
        nc.sync.dma_start(wob[:], wout_in.ap())
        onesd = pers.tile([128, 1], BF16, tag="onesd")    # 1/D for stats matmuls
        nc.vector.memset(onesd[:], 1.0 / D)
        epsc = pers.tile([128, 1], F32, tag="epsc")
        nc.vector.memset(epsc[:], LN_EPS)
        if not bw_zero:
            bwb = pers.tile([1, COLS], BF16, tag="bwb")
            nc.sync.dma_start(bwb[:], bw_in.ap())

        # ---- LN stats + qkv^T, per batch ----
        xpool = ctx.enter_context(tc.tile_pool(name="xT", bufs=1))
        tree = ctx.enter_context(tc.tile_pool(name="tree", bufs=1))
        rows = ctx.enter_context(tc.tile_pool(name="rows", bufs=2))
        rbc = ctx.enter_context(tc.tile_pool(name="rbc", bufs=3))
        x2p = ctx.enter_context(tc.tile_pool(name="x2p", bufs=2))
        with tc.tile_pool(name="pstat", bufs=1, space="PSUM") as pstat, \
             tc.tile_pool(name="pqkv", bufs=3, space="PSUM") as pqkv, \
             tc.tile_pool(name="pvt", bufs=2, space="PSUM") as pvt:
            for b in range(B):
                xb = xpool.tile([128, KS, N], BF16, tag=f"xb{b}", name=f"xb{b}")
                for k in range(KS):
                    nc.sync.dma_start(xb[:, k, :], xT_in.ap()[b, k * 128:(k + 1) * 128, :])
                nc.vector.memset(
                    vA[b][:, :, 64::65].rearrange("p t o -> p (t o)"), 1.0)
                dsts = (qT, kT, vT)
                for nt in range(IT):
                    sl = slice(nt * 512, (nt + 1) * 512)
                    # x^2 for all 8 slabs in one ScalarE pass
                    x2t = x2p.tile([128, KS, 512], BF16, tag="x2")
                    nc.scalar.activation(x2t[:], xb[:, :, sl], AF.Square)
                    # slab pre-sum trees (DVE strided adds, last level GpSimd)
                    s12 = tree.tile([128, 2, 512], BF16, tag="s12", bufs=2)
                    a1 = tree.tile([128, 4, 512], BF16, tag="a1")
                    c1 = tree.tile([128, 2, 512], BF16, tag="c1")
                    nc.vector.tensor_tensor(
                        a1[:], xb[:, 0::2, sl], xb[:, 1::2, sl], op=AL.add)
                    nc.vector.tensor_tensor(
                        c1[:], a1[:, 0::2, :], a1[:, 1::2, :], op=AL.add)
                    nc.gpsimd.tensor_tensor(
                        s12[:, 0, :], c1[:, 0, :], c1[:, 1, :], op=AL.add)
                    a2 = tree.tile([128, 4, 512], BF16, tag="a2")
                    c2 = tree.tile([128, 2, 512], BF16, tag="c2")
                    nc.vector.tensor_tensor(
                        a2[:], x2t[:, 0::2, :], x2t[:, 1::2, :], op=AL.add)
                    nc.vector.tensor_tensor(
                        c2[:], a2[:, 0::2, :], a2[:, 1::2, :], op=AL.add)
                    nc.gpsimd.tensor_tensor(
                        s12[:, 1, :], c2[:, 0, :], c2[:, 1, :], op=AL.add)
                    # partition reduction: one ones-vector matmul per stat
                    mu_ps = pstat.tile([1, 512], F32, tag="mu_ps")
                    nc.tensor.matmul(mu_ps[:], onesd[:], s12[:, 0, :],
                                     start=True, stop=True, **MM)
                    x2_ps = pstat.tile([1, 512], F32, tag="x2_ps")
                    nc.tensor.matmul(x2_ps[:], onesd[:], s12[:, 1, :],
                                     start=True, stop=True, **MM)
                    # row chain: var = E[x^2] - mu^2, rsig = 1/sqrt(var+eps)
                    mu2_r = rows.tile([1, 512], F32, tag="mu2_r")
                    nc.scalar.activation(mu2_r[:], mu_ps[:], AF.Square)
                    var_r = rows.tile([1, 512], F32, tag="var_r")
                    nc.vector.tensor_tensor(var_r[:], x2_ps[:], mu2_r[:],
                                            op=AL.subtract)
                    sd_r = rows.tile([1, 512], F32, tag="sd_r")
                    nc.scalar.activation(sd_r[:], var_r[:], AF.Sqrt,
                                         bias=epsc[0:1, :])
                    rsig_r = rows.tile([1, 512], F32, tag="rsig_r")
                    nc.vector.reciprocal_approx_fast(rsig_r[:], sd_r[:])
                    rsig_bf = rows.tile([1, 512], BF16, tag="rsig_bf")
                    nc.gpsimd.tensor_copy(rsig_bf[:], rsig_r[:])
                    rsig_bc = rbc.tile([128, 512], BF16, tag="rsbc")
                    nc.gpsimd.partition_broadcast(rsig_bc[:], rsig_bf[:],
                                                  channels=128)
                    if debug:
                        nc.sync.dma_start(drs_out.ap()[b, :, sl], rsig_bc[:])
                    if not bw_zero:
                        sd_bf = rows.tile([1, 512], BF16, tag="sd_bf")
                        nc.vector.tensor_copy(sd_bf[:], sd_r[:])
                    # qkv projection (mu correction folded into wqb on host)
                    for blk in range(3):
                        csl = slice(blk * 128, (blk + 1) * 128)
                        ps = pqkv.tile([128, 512], F32, tag="psqkv")
                        for k in range(KS):
                            nc.tensor.matmul(ps[:], wqb[:, k, csl], xb[:, k, sl],
                                             start=(k == 0),
                                             stop=(k == KS - 1 and bw_zero), **MM)
                        if not bw_zero:
                            nc.tensor.matmul(ps[:], bwb[0:1, csl], sd_bf[0:1, :],
                                             start=False, stop=True, **MM)
                        nc.vector.tensor_tensor(dsts[blk][b][:, sl], ps[:],
                                                rsig_bc[:], op=AL.mult)
                    # V -> natural layout (ones column pre-memset), 4 PE
                    # transposes batched into one PSUM tile + one ScalarE copy
                    pst = pvt.tile([128, 4, 128], BF16, tag="pst")
                    for i, t in enumerate(range(4 * nt, 4 * nt + 4)):
                        nc.tensor.transpose(
                            pst[:, i, :], vT[b][:, t * 128:(t + 1) * 128], ident[:])
                    nc.scalar.copy(
                        vA[b][:, 4 * nt:4 * nt + 4, :]
                        .rearrange("p t (h v) -> p t h v", h=2)[:, :, :, 0:64],
                        pst[:].rearrange("p t (h v) -> p t h v", h=2))
                if debug:
                    for blk2, srcT in enumerate((qT, kT, vT)):
                        nc.sync.dma_start(dq_out.ap()[b, blk2], srcT[b][:])
                    nc.sync.dma_start(
                        dva_out.ap()[b],
                        vA[b][:].rearrange("p t o -> p (t o)"))

        # ---- attention + interleaved out-projection ----
        with tc.tile_pool(name="bias", bufs=6) as bias_pool, \
             tc.tile_pool(name="pexp", bufs=4) as exp_pool, \
             tc.tile_pool(name="lnrm", bufs=2) as lnrm, \
             tc.tile_pool(name="ysb", bufs=2) as ysb, \
             tc.tile_pool(name="scp", bufs=2, space="PSUM") as scp, \
             tc.tile_pool(name="psop", bufs=1, space="PSUM") as psop:
            for t in range(IT):
                isl = slice(t * 512, (t + 1) * 512)
                nj = 4 * (t + 1)
                pso = {(b, h): psop.tile([65, 512], F32, tag=f"pso{b}{h}",
                                         name=f"pso{b}{h}")
                       for b in range(B) for h in range(HL)}
                for g in _attn_groups(t, pair_exp):
                    bts = []
                    for h in range(HL):
                        bt = bias_pool.tile([128, 2, 512], BF16, tag="bt")
                        for gi, (j, off, _) in enumerate(g):
                            nc.sync.dma_start(
                                bt[:, gi, off:],
                                biasT_in.ap()[h, j * 128:(j + 1) * 128,
                                              t * 512 + off:(t + 1) * 512])
                        bts.append(bt)
                    for b in range(B):
                        sc = [scp.tile([128, 2, 512], F32, tag="scps",
                                       name=f"sc{h}") for h in range(HL)]
                        # scores: C=64 head pair -> PE row groups (0,0)/(64,0)
                        for gi, (j, off, _) in enumerate(g):
                            for h in range(HL):
                                hsl = slice(h * 64, (h + 1) * 64)
                                nc.tensor.matmul(
                                    sc[h][:, gi, off:],
                                    kT[b][hsl, j * 128:(j + 1) * 128],
                                    qT[b][hsl, t * 512 + off:(t + 1) * 512],
                                    start=True, stop=False, **MM)
                        # bias add: two 64x64 identity quadrants, concurrent
                        for gi, (j, off, _) in enumerate(g):
                            for h in range(HL):
                                if quad_bias:
                                    nc.tensor.matmul(
                                        sc[h][0:64, gi, off:], ident[0:64, 0:64],
                                        bts[h][0:64, gi, off:],
                                        start=False, stop=True, **MM)
                                    nc.tensor.matmul(
                                        sc[h][64:128, gi, off:], ident[64:128, 64:128],
                                        bts[h][64:128, gi, off:],
                                        start=False, stop=True, **MM)
                                else:
                                    nc.tensor.matmul(
                                        sc[h][:, gi, off:], ident[:],
                                        bts[h][:, gi, off:],
                                        start=False, stop=True, **MM)
                        for h in range(HL):
                            pe = exp_pool.tile([128, 2, 512], BF16, tag="pe")
                            if len(g) == 2:
                                nc.scalar.activation(pe[:], sc[h][:], AF.Exp)
                            else:
                                off = g[0][1]
                                nc.scalar.activation(pe[:, 0, off:],
                                                     sc[h][:, 0, off:], AF.Exp)
                            for gi, (j, _, pv_off) in enumerate(g):
                                nc.tensor.matmul(
                                    pso[(b, h)][:, pv_off:],
                                    vA[b][:, j, h * 65:h * 65 + 65],
                                    pe[:, gi, pv_off:],
                                    start=(j == 0), stop=(j == nj - 1), **MM)
                # deferred softmax normalization + eviction to oT
                for b in range(B):
                    for h in range(HL):
                        lrow = lnrm.tile([1, 512], F32, tag="lrow")
                        nc.vector.tensor_copy(lrow[:], pso[(b, h)][64:65, :])
                        rec = lnrm.tile([1, 512], F32, tag="rec")
                        nc.vector.reciprocal_approx_fast(rec[:], lrow[:])
                        rec_bf = lnrm.tile([1, 512], BF16, tag="rec_bf")
                        nc.gpsimd.tensor_copy(rec_bf[:], rec[:])
                        if debug:
                            nc.sync.dma_start(drec_out.ap()[b, h, t], rec_bf[:])
                        lb = lnrm.tile([64, 512], BF16, tag="lb")
                        nc.gpsimd.partition_broadcast(lb[:], rec_bf[:],
                                                      channels=64)
                        nc.vector.tensor_tensor(
                            oT[b][h * 64:(h + 1) * 64, isl],
                            pso[(b, h)][0:64, :], lb[:], op=AL.mult)
                if debug:
                    for b in range(B):
                        nc.sync.dma_start(do_out.ap()[b, :, isl], oT[b][:, isl])
                # out-projection for this i-tile (psum shared with scores)
                for b in range(B):
                    for tt in range(4 * t, 4 * t + 4):
                        psy = scp.tile([128, 2, 512], F32, tag="scps", name="psy")
                        for half in range(2):
                            nc.tensor.matmul(psy[:, half, :],
                                             oT[b][:, tt * 128:(tt + 1) * 128],
                                             wob[:, half * 512:(half + 1) * 512],
                                             start=True, stop=True, **MM)
                        yt = ysb.tile([128, D], BF16, tag="yt")
                        nc.vector.tensor_copy(
                            yt[:], psy[:].rearrange("p a b -> p (a b)"))
                        nc.sync.dma_start(y_out.ap()[b, tt * 128:(tt + 1) * 128, :],
                                          yt[:])

    nc.compile()
    return nc


_NC_CACHE = {}


def _get_program(bw_zero=True):
    if bw_zero not in _NC_CACHE:
        _NC_CACHE[bw_zero] = build_program(bw_zero)
    return _NC_CACHE[bw_zero]


def build_in_maps(x, attn_bias, ln_gamma, ln_beta, w_qkv, w_out):
    x = np.asarray(x, dtype=np.float32)
    attn_bias = np.asarray(attn_bias, dtype=np.float32)
    ln_gamma = np.asarray(ln_gamma, dtype=np.float32)
    ln_beta = np.asarray(ln_beta, dtype=np.float32)
    w_qkv = np.asarray(w_qkv, dtype=np.float32)
    w_out = np.asarray(w_out, dtype=np.float32)

    ident = np.eye(128, dtype=ml_dtypes.bfloat16)
    xT = np.ascontiguousarray(x.transpose(0, 2, 1)).astype(ml_dtypes.bfloat16)
    # causal mask folded into the bias, transposed to [head, key j, query i]
    tri = np.triu(np.ones((N, N), dtype=bool), k=1)  # True above diag (masked)
    in_maps = []
    for c in range(N_CORES):
        h0 = HL * c
        cols = np.concatenate([
            w_qkv[:, q * H * DH + h0 * DH: q * H * DH + (h0 + HL) * DH]
            for q in range(3)], axis=1)
        # gamma scaling + attention scale on the q block
        cols = cols * ln_gamma[:, None]
        cols[:, 0:128] *= SCALE
        # beta @ W row (before mean-fold; the fold cancels against mu anyway)
        bw = (ln_beta @ cols)[None, :]
        # fold the LN mean subtraction into the weights:
        # x^T (W - 1 colsum(W)/D) = (x - mu)^T W
        cols = cols - cols.sum(axis=0, keepdims=True) / D
        biasT = np.empty((HL, N, N), dtype=ml_dtypes.bfloat16)
        for h in range(HL):
            bh = attn_bias[h0 + h].copy()
            bh[tri] = NEG
            biasT[h] = bh.T.astype(ml_dtypes.bfloat16)
        in_maps.append({
            "xT": xT,
            "biasT": biasT,
            "wqkv": np.ascontiguousarray(cols).astype(ml_dtypes.bfloat16),
            "wout": np.ascontiguousarray(
                w_out[h0 * DH:(h0 + HL) * DH]).astype(ml_dtypes.bfloat16),
            "bw": bw.astype(ml_dtypes.bfloat16),
            "ident": ident,
        })
    return in_maps


def kernel(x, attn_bias, ln_gamma, ln_beta, w_qkv, w_out):
    in_maps = build_in_maps(x, attn_bias, ln_gamma, ln_beta, w_qkv, w_out)
    bw_zero = all(np.all(m["bw"] == 0) for m in in_maps)
    nc = _get_program(bw_zero)
    res = run_bass_kernel_spmd(nc, in_maps, core_ids=list(range(N_CORES)))
    out = np.zeros((B, N, D), dtype=np.float32)
    for c in range(N_CORES):
        out += res.results[c]["y"].astype(np.float32)
    return out


# revision 17
# speedup vs baseline: 1.1275x; 1.1275x over previous
"""Fused LayerNorm + causal multi-head attention (with additive bias) + out-proj
for Trainium2, SPMD over 8 NeuronCores.

Sharding: tensor-parallel over heads. 16 heads / 8 cores = 2 heads per core.
Each core computes LN(x) (replicated), the qkv projection restricted to its
2 heads' columns, causal softmax attention with its heads' bias slices, and a
partial output projection (its heads' rows of w_out). Host sums the 8 partial
outputs (the TP all-reduce, done on gather) in f32 from bf16 partials.

Key layout/algorithm choices (v2, tuned from the v1 trace):
 - x arrives pre-transposed ([dim, token], bf16). The LN mean-subtraction is
   folded into the weights on the HOST: W'' = gammaW - (1/D) 1 colsum(gammaW),
   so x^T W'' = (x - mu)^T gammaW directly. Only rsig (1/std) is applied on
   device, riding the PSUM->SBUF eviction (DVE multiply against a broadcast
   rsig row). The beta term is a rank-1 PSUM matmul (skipped when beta == 0).
 - LN variance stats: slab pre-sums of x and x^2 run on the DVE as two
   strided tensor-tensor adds per level (one instruction does 4 adds), the
   last level on GpSimd; the 128-partition reduction is a single ones-vector
   matmul per 512-token block (PE cost 8k cycles vs 65k for direct matmuls).
   x^2 is produced by one ScalarE Square over [128, 8, 512] per block.
 - Scores are computed transposed, S^T[j, i] = (k_j . q_i); the two heads'
   C=64 matmuls auto-pair into PE row-groups (0,0)/(64,0) and run
   concurrently.
 - The causal mask and softmax max-subtraction are folded into the host bias
   (pre-masked with -1e9; logits are O(10) so exp never overflows).
 - The bias add runs on the PE as TWO 64x64-quadrant identity matmuls
   (quadrants (0,0) and (64,64)) which also run concurrently - half the PE
   cost of a full 128-contraction identity accumulate.
 - exp() instructions are the attention bottleneck (~293ns fixed overhead
   each); score PSUM tiles are [128, 2, 512] spanning two banks so one
   ACTIVATE covers two j-tiles. The first diagonal j-tile is computed
   full-width (its masked region gets -1e9 from the bias, exp -> 0) so it can
   pair with its predecessor; the remaining two diagonal tiles are trimmed
   singles.
 - Softmax normalization is deferred: an all-ones column appended to V gives
   the row sums l_i for free; 1/l is applied to O^T after P@V.
 - The out-projection is interleaved per 512-token i-tile right after its
   attention completes, sharing the score PSUM pool, so the y writeback DMA
   (bf16) overlaps attention compute instead of trailing the kernel.
"""

import numpy as np
import ml_dtypes
from contextlib import ExitStack

import concourse.bass as bass
import concourse.tile as tile
from concourse import bacc, mybir
from concourse.bass_utils import run_bass_kernel_spmd

F32 = mybir.dt.float32
BF16 = mybir.dt.bfloat16
AL = mybir.AluOpType
AF = mybir.ActivationFunctionType

N_CORES = 8
B = 2            # batch
N = 2048         # tokens
D = 1024         # model dim
H = 16           # total heads
HL = 2           # heads per core
DH = 64          # head dim
COLS = 3 * HL * DH   # 384 qkv columns per core
KS = D // 128    # 8 contraction slabs
TT = N // 128    # 16 token tiles
IT = N // 512    # 4 i-tiles (query tiles of 512)
SCALE = DH ** -0.5
LN_EPS = 1e-5
NEG = -1.0e9


def build_program(bw_zero=True, debug=False):
    nc = bacc.Bacc("TRN2", target_bir_lowering=False, debug=False)

    xT_in = nc.dram_tensor("xT", [B, D, N], BF16, kind="ExternalInput")
    biasT_in = nc.dram_tensor("biasT", [HL, N, N], BF16, kind="ExternalInput")
    wqkv_in = nc.dram_tensor("wqkv", [D, COLS], BF16, kind="ExternalInput")
    wout_in = nc.dram_tensor("wout", [HL * DH, D], BF16, kind="ExternalInput")
    bw_in = nc.dram_tensor("bw", [1, COLS], BF16, kind="ExternalInput")
    ident_in = nc.dram_tensor("ident", [128, 128], BF16, kind="ExternalInput")
    y_out = nc.dram_tensor("y", [B, N, D], BF16, kind="ExternalOutput")
    if debug:
        dq_out = nc.dram_tensor("dq", [B, 3, 128, N], BF16, kind="ExternalOutput")
        drs_out = nc.dram_tensor("drs", [B, 128, N], F32, kind="ExternalOutput")
        do_out = nc.dram_tensor("do", [B, 128, N], BF16, kind="ExternalOutput")
        dva_out = nc.dram_tensor("dva", [B, 128, TT * 130], BF16, kind="ExternalOutput")
        drec_out = nc.dram_tensor("drec", [B, HL, IT, 512], F32, kind="ExternalOutput")

    MM = dict(skip_group_check=True)

    with tile.TileContext(nc) as tc, ExitStack() as ctx:
        # ---- persistent sbuf ----
        pers = ctx.enter_context(tc.tile_pool(name="pers", bufs=1))
        qT = [pers.tile([128, N], BF16, tag=f"qT{b}", name=f"qT{b}") for b in range(B)]
        kT = [pers.tile([128, N], BF16, tag=f"kT{b}", name=f"kT{b}") for b in range(B)]
        vT = [pers.tile([128, N], BF16, tag=f"vT{b}", name=f"vT{b}") for b in range(B)]
        # V natural with ones column: per key-tile [.., 130]: h0 v(64)+1, h1 v(64)+1
        vA = [pers.tile([128, TT, 130], BF16, tag=f"vA{b}", name=f"vA{b}") for b in range(B)]
        oT = [pers.tile([128, N], BF16, tag=f"oT{b}", name=f"oT{b}") for b in range(B)]
        ident = pers.tile([128, 128], BF16, tag="ident")
        nc.sync.dma_start(ident[:], ident_in.ap())
        wqb = pers.tile([128, KS, COLS], BF16, tag="wqb")
        nc.sync.dma_start(wqb[:], wqkv_in.ap().rearrange("(k p) c -> p k c", p=128))
        wob = pers.tile([128, D], BF16, tag="wob")
        nc.sync.dma_start(wob[:], wout_in.ap())
        onesd = pers.tile([128, 1], BF16, tag="onesd")    # 1/D for stats matmuls
        nc.vector.memset(onesd[:], 1.0 / D)
        epsc = pers.tile([128, 1], F32, tag="epsc")
        nc.vector.memset(epsc[:], LN_EPS)
        if not bw_zero:
            bwb = pers.tile([1, COLS], BF16, tag="bwb")
            nc.sync.dma_start(bwb[:], bw_in.ap())

        # ---- LN stats + qkv^T, per batch ----
        xpool = ctx.enter_context(tc.tile_pool(name="xT", bufs=1))
        tree = ctx.enter_context(tc.tile_pool(name="tree", bufs=2))
        rows = ctx.enter_context(tc.tile_pool(name="rows", bufs=2))
        rbc = ctx.enter_context(tc.tile_pool(name="rbc", bufs=3))
        x2p = ctx.enter_context(tc.tile_pool(name="x2p", bufs=2))
        with tc.tile_pool(name="pstat", bufs=2, space="PSUM") as pstat, \
             tc.tile_pool(name="pqkv", bufs=3, space="PSUM") as pqkv, \
             tc.tile_pool(name="pvt", bufs=1, space="PSUM") as pvt:
            for b in range(B):
                xb = xpool.tile([128, KS, N], BF16, tag=f"xb{b}", name=f"xb{b}")
                for k in range(KS):
                    nc.sync.dma_start(xb[:, k, :], xT_in.ap()[b, k * 128:(k + 1) * 128, :])
                nc.vector.memset(
                    vA[b][:, :, 64::65].rearrange("p t o -> p (t o)"), 1.0)
                dsts = (qT, kT, vT)
                for nt in range(IT):
                    sl = slice(nt * 512, (nt + 1) * 512)
                    # x^2 for all 8 slabs in one ScalarE pass
                    x2t = x2p.tile([128, KS, 512], BF16, tag="x2")
                    nc.scalar.activation(x2t[:], xb[:, :, sl], AF.Square)
                    # slab pre-sum trees (DVE strided adds, last level GpSimd)
                    s12 = tree.tile([128, 2, 512], BF16, tag="s12", bufs=2)
                    a1 = tree.tile([128, 4, 512], BF16, tag="a1")
                    c1 = tree.tile([128, 2, 512], BF16, tag="c1")
                    nc.vector.tensor_tensor(
                        a1[:], xb[:, 0::2, sl], xb[:, 1::2, sl], op=AL.add)
                    nc.vector.tensor_tensor(
                        c1[:], a1[:, 0::2, :], a1[:, 1::2, :], op=AL.add)
                    nc.gpsimd.tensor_tensor(
                        s12[:, 0, :], c1[:, 0, :], c1[:, 1, :], op=AL.add)
                    a2 = tree.tile([128, 4, 512], BF16, tag="a2")
                    c2 = tree.tile([128, 2, 512], BF16, tag="c2")
                    nc.vector.tensor_tensor(
                        a2[:], x2t[:, 0::2, :], x2t[:, 1::2, :], op=AL.add)
                    nc.vector.tensor_tensor(
                        c2[:], a2[:, 0::2, :], a2[:, 1::2, :], op=AL.add)
                    nc.gpsimd.tensor_tensor(
                        s12[:, 1, :], c2[:, 0, :], c2[:, 1, :], op=AL.add)
                    # partition reduction: one ones-vector matmul per stat
                    mu_ps = pstat.tile([1, 512], F32, tag="mu_ps")
                    nc.tensor.matmul(mu_ps[:], onesd[:], s12[:, 0, :],
                                     start=True, stop=True, **MM)
                    x2_ps = pstat.tile([1, 512], F32, tag="x2_ps")
                    nc.tensor.matmul(x2_ps[:], onesd[:], s12[:, 1, :],
                                     start=True, stop=True, **MM)
                    # row chain: var = E[x^2] - mu^2, rsig = 1/sqrt(var+eps)
                    mu2_r = rows.tile([1, 512], F32, tag="mu2_r", bufs=1)
                    nc.scalar.activation(mu2_r[:], mu_ps[:], AF.Square)
                    var_r = rows.tile([1, 512], F32, tag="var_r", bufs=1)
                    nc.vector.tensor_tensor(var_r[:], x2_ps[:], mu2_r[:],
                                            op=AL.subtract)
                    sd_r = rows.tile([1, 512], F32, tag="sd_r")
                    nc.scalar.activation(sd_r[:], var_r[:], AF.Sqrt,
                                         bias=epsc[0:1, :])
                    rsig_r = rows.tile([1, 512], F32, tag="rsig_r")
                    nc.vector.reciprocal_approx_fast(rsig_r[:], sd_r[:])
                    rsig_bc = rbc.tile([128, 512], F32, tag="rsbc")
                    nc.gpsimd.partition_broadcast(rsig_bc[:], rsig_r[:],
                                                  channels=128)
                    if debug:
                        nc.sync.dma_start(drs_out.ap()[b, :, sl], rsig_bc[:])
                    if not bw_zero:
                        sd_bf = rows.tile([1, 512], BF16, tag="sd_bf")
                        nc.vector.tensor_copy(sd_bf[:], sd_r[:])
                    # qkv projection (mu correction folded into wqb on host)
                    for blk in range(3):
                        csl = slice(blk * 128, (blk + 1) * 128)
                        ps = pqkv.tile([128, 512], F32, tag="psqkv")
                        for k in range(KS):
                            nc.tensor.matmul(ps[:], wqb[:, k, csl], xb[:, k, sl],
                                             start=(k == 0),
                                             stop=(k == KS - 1 and bw_zero), **MM)
                        if not bw_zero:
                            nc.tensor.matmul(ps[:], bwb[0:1, csl], sd_bf[0:1, :],
                                             start=False, stop=True, **MM)
                        nc.vector.tensor_tensor(dsts[blk][b][:, sl], ps[:],
                                                rsig_bc[:], op=AL.mult)
                    # V -> natural layout (ones column pre-memset), 4 PE
                    # transposes batched into one PSUM tile + one ScalarE copy
                    pst = pvt.tile([128, 4, 128], BF16, tag="pst")
                    for i, t in enumerate(range(4 * nt, 4 * nt + 4)):
                        nc.tensor.transpose(
                            pst[:, i, :], vT[b][:, t * 128:(t + 1) * 128], ident[:])
                    nc.scalar.copy(
                        vA[b][:, 4 * nt:4 * nt + 4, :]
                        .rearrange("p t (h v) -> p t h v", h=2)[:, :, :, 0:64],
                        pst[:].rearrange("p t (h v) -> p t h v", h=2))
                if debug:
                    for blk2, srcT in enumerate((qT, kT, vT)):
                        nc.sync.dma_start(dq_out.ap()[b, blk2], srcT[b][:])
                    nc.sync.dma_start(
                        dva_out.ap()[b],
                        vA[b][:].rearrange("p t o -> p (t o)"))

        # ---- attention + interleaved out-projection ----
        # h is the outer loop so only one head's pso pair (2 banks) is live,
        # leaving 3 two-bank score slots: the PE pipelines across exp()
        # latencies. Each PSUM bank holds exactly one accumulation group.
        with tc.tile_pool(name="bias", bufs=5) as bias_pool, \
             tc.tile_pool(name="pexp", bufs=3) as exp_pool, \
             tc.tile_pool(name="lnrm", bufs=2) as lnrm, \
             tc.tile_pool(name="ysb", bufs=2) as ysb, \
             tc.tile_pool(name="scp", bufs=3, space="PSUM") as scp, \
             tc.tile_pool(name="psop", bufs=1, space="PSUM") as psop:
            for t in range(IT):
                isl = slice(t * 512, (t + 1) * 512)
                nj = 4 * (t + 1)
                # groups of (j, off): pairs over j=0..4t+1 (the diagonal tile
                # 4t+1 computed full width - host bias -1e9 masks it), then
                # two trimmed singles
                groups = [[(2 * k, 0), (2 * k + 1, 0)] for k in range(2 * t + 1)]
                groups.append([(4 * t + 2, 256)])
                groups.append([(4 * t + 3, 384)])
                for h in range(HL):
                    hsl = slice(h * 64, (h + 1) * 64)
                    pso = {b: psop.tile([65, 512], F32, tag=f"pso{b}",
                                        name=f"pso{b}") for b in range(B)}
                    for g in groups:
                        bt = bias_pool.tile([128, 2, 512], BF16, tag="bt")
                        for si, (j, off) in enumerate(g):
                            nc.sync.dma_start(
                                bt[:, si, off:],
                                biasT_in.ap()[h, j * 128:(j + 1) * 128,
                                              t * 512 + off:(t + 1) * 512])
                        for b in range(B):
                            sc = scp.tile([128, 2, 512], F32, tag="scps",
                                          name="sc")
                            for si, (j, off) in enumerate(g):
                                nc.tensor.matmul(
                                    sc[:, si, off:],
                                    kT[b][hsl, j * 128:(j + 1) * 128],
                                    qT[b][hsl, t * 512 + off:(t + 1) * 512],
                                    start=True, stop=False, **MM)
                            # bias add: two 64x64 identity quadrants (the lo/hi
                            # pair runs concurrently in disjoint PE quadrants)
                            for si, (j, off) in enumerate(g):
                                nc.tensor.matmul(
                                    sc[0:64, si, off:], ident[0:64, 0:64],
                                    bt[0:64, si, off:],
                                    start=False, stop=True, **MM)
                                nc.tensor.matmul(
                                    sc[64:128, si, off:], ident[64:128, 64:128],
                                    bt[64:128, si, off:],
                                    start=False, stop=True, **MM)
                            pe = exp_pool.tile([128, 2, 512], BF16, tag="pe")
                            if len(g) == 2:
                                nc.scalar.activation(pe[:], sc[:], AF.Exp)
                            else:
                                off = g[0][1]
                                nc.scalar.activation(pe[:, 0, off:],
                                                     sc[:, 0, off:], AF.Exp)
                            for si, (j, off) in enumerate(g):
                                pv_off = max(0, 128 * j - 512 * t)
                                nc.tensor.matmul(
                                    pso[b][:, pv_off:],
                                    vA[b][:, j, h * 65:h * 65 + 65],
                                    pe[:, si, pv_off:],
                                    start=(j == 0), stop=(j == nj - 1), **MM)
                    # deferred softmax normalization + eviction to oT
                    for b in range(B):
                        lrow = lnrm.tile([1, 512], F32, tag="lrow", bufs=1)
                        nc.vector.tensor_copy(lrow[:], pso[b][64:65, :])
                        rec = lnrm.tile([1, 512], F32, tag="rec")
                        nc.vector.reciprocal_approx_fast(rec[:], lrow[:])
                        if debug:
                            nc.sync.dma_start(drec_out.ap()[b, h, t], rec[:])
                        lb = lnrm.tile([64, 512], F32, tag="lb")
                        nc.gpsimd.partition_broadcast(lb[:], rec[:], channels=64)
                        nc.vector.tensor_tensor(
                            oT[b][hsl, isl], pso[b][0:64, :], lb[:], op=AL.mult)
                if debug:
                    for b in range(B):
                        nc.sync.dma_start(do_out.ap()[b, :, isl], oT[b][:, isl])
                # out-projection for this i-tile (psum shared with scores)
                for b in range(B):
                    for tt in range(4 * t, 4 * t + 4):
                        psy = scp.tile([128, 2, 512], F32, tag="scps", name="psy")
                        for half in range(2):
                            nc.tensor.matmul(psy[:, half, :],
                                             oT[b][:, tt * 128:(tt + 1) * 128],
                                             wob[:, half * 512:(half + 1) * 512],
                                             start=True, stop=True, **MM)
                        yt = ysb.tile([128, D], BF16, tag="yt")
                        nc.vector.tensor_copy(
                            yt[:], psy[:].rearrange("p a b -> p (a b)"))
                        nc.sync.dma_start(y_out.ap()[b, tt * 128:(tt + 1) * 128, :],
                                          yt[:])

    nc.compile()
    return nc


_NC_CACHE = {}


def _get_program(bw_zero=True):
    if bw_zero not in _NC_CACHE:
        _NC_CACHE[bw_zero] = build_program(bw_zero)
    return _NC_CACHE[bw_zero]


def build_in_maps(x, attn_bias, ln_gamma, ln_beta, w_qkv, w_out):
    x = np.asarray(x, dtype=np.float32)
    attn_bias = np.asarray(attn_bias, dtype=np.float32)
    ln_gamma = np.asarray(ln_gamma, dtype=np.float32)
    ln_beta = np.asarray(ln_beta, dtype=np.float32)
    w_qkv = np.asarray(w_qkv, dtype=np.float32)
    w_out = np.asarray(w_out, dtype=np.float32)

    ident = np.eye(128, dtype=ml_dtypes.bfloat16)
    xT = np.ascontiguousarray(x.transpose(0, 2, 1)).astype(ml_dtypes.bfloat16)
    # causal mask folded into the bias, transposed to [head, key j, query i]
    tri = np.triu(np.ones((N, N), dtype=bool), k=1)  # True above diag (masked)
    in_maps = []
    for c in range(N_CORES):
        h0 = HL * c
        cols = np.concatenate([
            w_qkv[:, q * H * DH + h0 * DH: q * H * DH + (h0 + HL) * DH]
            for q in range(3)], axis=1)
        # gamma scaling + attention scale on the q block
        cols = cols * ln_gamma[:, None]
        cols[:, 0:128] *= SCALE
        # beta @ W row (before mean-fold; the fold cancels against mu anyway)
        bw = (ln_beta @ cols)[None, :]
        # fold the LN mean subtraction into the weights:
        # x^T (W - 1 colsum(W)/D) = (x - mu)^T W
        cols = cols - cols.sum(axis=0, keepdims=True) / D
        biasT = np.empty((HL, N, N), dtype=ml_dtypes.bfloat16)
        for h in range(HL):
            bh = attn_bias[h0 + h].copy()
            bh[tri] = NEG
            biasT[h] = bh.T.astype(ml_dtypes.bfloat16)
        in_maps.append({
            "xT": xT,
            "biasT": biasT,
            "wqkv": np.ascontiguousarray(cols).astype(ml_dtypes.bfloat16),
            "wout": np.ascontiguousarray(
                w_out[h0 * DH:(h0 + HL) * DH]).astype(ml_dtypes.bfloat16),
            "bw": bw.astype(ml_dtypes.bfloat16),
            "ident": ident,
        })
    return in_maps


def kernel(x, attn_bias, ln_gamma, ln_beta, w_qkv, w_out):
    in_maps = build_in_maps(x, attn_bias, ln_gamma, ln_beta, w_qkv, w_out)
    bw_zero = all(np.all(m["bw"] == 0) for m in in_maps)
    nc = _get_program(bw_zero)
    res = run_bass_kernel_spmd(nc, in_maps, core_ids=list(range(N_CORES)))
    out = np.zeros((B, N, D), dtype=np.float32)
    for c in range(N_CORES):
        out += res.results[c]["y"].astype(np.float32)
    return out


# revision 19
# speedup vs baseline: 1.2113x; 1.0743x over previous
"""Fused LayerNorm + causal multi-head attention (with additive bias) + out-proj
for Trainium2, SPMD over 8 NeuronCores.

Sharding: tensor-parallel over heads. 16 heads / 8 cores = 2 heads per core.
Each core computes LN(x) (replicated), the qkv projection restricted to its
2 heads' columns, causal softmax attention with its heads' bias slices, and a
partial output projection (its heads' rows of w_out). Host sums the 8 partial
outputs (the TP all-reduce, done on gather) in f32 from bf16 partials.

Key choices (engine queues are strict FIFO, so emission order is software-
pipelined to keep the PE fed):
 - x arrives pre-transposed ([dim, token], bf16). The LN mean-subtraction is
   folded into the weights on the HOST: W'' = gW - (1/D) 1 colsum(gW), so
   x^T W'' = (x - mu)^T gW directly. Only rsig (1/std) is applied on device,
   riding the PSUM->SBUF eviction (DVE multiply against a broadcast f32 rsig
   row). The beta term is a rank-1 PSUM matmul (skipped when beta == 0).
 - LN variance stats: slab pre-sums of x and x^2 run on the DVE as strided
   tensor-tensor adds (one instruction does 4 adds), last level on GpSimd;
   the 128-partition reduction is one ones-vector matmul per 512-token block
   (8k PE cycles total vs 65k for direct stat matmuls). x^2 comes from one
   ScalarE Square over [128, 8, 512] per block. Stage emission is pipelined:
   squares/trees for block nt+1 are emitted before block nt's consumers, and
   the V transposes for nt-1 after block nt's qkv matmuls, so no engine FIFO
   head-of-line blocks on a cross-engine chain.
 - Scores are computed transposed, S^T[j, i] = (k_j . q_i); the two heads'
   C=64 matmuls are emitted back-to-back and auto-pair into PE row groups
   (0,0)/(64,0), running concurrently.
 - The causal mask and softmax max-subtraction are folded into the host bias
   (pre-masked with -1e9; logits are O(10) so exp never overflows). The
   diagonal-adjacent j-tile is computed full width so it pairs with its
   neighbour in one exp; only fully-masked regions of the two outer diagonal
   tiles are trimmed.
 - Score PSUM tiles span two banks ([128, 2, 512]) so one exp ACTIVATE covers
   two j-tiles (~293ns fixed cost per ACTIVATE). Batch is the OUTER attention
   loop so only one head-pair of P@V accumulators is live (2 banks; a PSUM
   bank supports exactly one accumulation group), leaving 3 score slots; P@V
   matmuls are emitted one (group, head-pair) late so the PE never stalls on
   an exp.
 - Softmax normalization is deferred: an all-ones column appended to V gives
   the row sums l_i for free; 1/l is applied to O^T after P@V.
 - The out-projection for each 512-token i-tile is emitted one loop step
   late, interleaving with the next tile's attention; y is written bf16.
"""

import numpy as np
import ml_dtypes
from contextlib import ExitStack

import concourse.bass as bass
import concourse.tile as tile
from concourse import bacc, mybir
from concourse.bass_utils import run_bass_kernel_spmd

F32 = mybir.dt.float32
BF16 = mybir.dt.bfloat16
AL = mybir.AluOpType
AF = mybir.ActivationFunctionType

N_CORES = 8
B = 2            # batch
N = 2048         # tokens
D = 1024         # model dim
H = 16           # total heads
HL = 2           # heads per core
DH = 64          # head dim
COLS = 3 * HL * DH   # 384 qkv columns per core
KS = D // 128    # 8 contraction slabs
TT = N // 128    # 16 token tiles
IT = N // 512    # 4 i-tiles (query tiles of 512)
SCALE = DH ** -0.5
LN_EPS = 1e-5
NEG = -1.0e9


def build_program(bw_zero=True, debug=False):
    nc = bacc.Bacc("TRN2", target_bir_lowering=False, debug=False)

    xT_in = nc.dram_tensor("xT", [B, D, N], BF16, kind="ExternalInput")
    biasT_in = nc.dram_tensor("biasT", [HL, N, N], BF16, kind="ExternalInput")
    wqkv_in = nc.dram_tensor("wqkv", [D, COLS], BF16, kind="ExternalInput")
    wout_in = nc.dram_tensor("wout", [HL * DH, D], BF16, kind="ExternalInput")
    bw_in = nc.dram_tensor("bw", [1, COLS], BF16, kind="ExternalInput")
    ident_in = nc.dram_tensor("ident", [128, 128], BF16, kind="ExternalInput")
    y_out = nc.dram_tensor("y", [B, N, D], BF16, kind="ExternalOutput")
    if debug:
        dq_out = nc.dram_tensor("dq", [B, 3, 128, N], BF16, kind="ExternalOutput")
        drs_out = nc.dram_tensor("drs", [B, 128, N], F32, kind="ExternalOutput")
        do_out = nc.dram_tensor("do", [B, 128, N], BF16, kind="ExternalOutput")
        dva_out = nc.dram_tensor("dva", [B, 128, TT * 130], BF16, kind="ExternalOutput")
        drec_out = nc.dram_tensor("drec", [B, HL, IT, 512], F32, kind="ExternalOutput")

    MM = dict(skip_group_check=True)

    with tile.TileContext(nc) as tc, ExitStack() as ctx:
        # ---- persistent sbuf ----
        pers = ctx.enter_context(tc.tile_pool(name="pers", bufs=1))
        qT = [pers.tile([128, N], BF16, tag=f"qT{b}", name=f"qT{b}") for b in range(B)]
        kT = [pers.tile([128, N], BF16, tag=f"kT{b}", name=f"kT{b}") for b in range(B)]
        vT = [pers.tile([128, N], BF16, tag=f"vT{b}", name=f"vT{b}") for b in range(B)]
        # V natural with ones column: per key-tile [.., 130]: h0 v(64)+1, h1 v(64)+1
        vA = [pers.tile([128, TT, 130], BF16, tag=f"vA{b}", name=f"vA{b}") for b in range(B)]
        oT = [pers.tile([128, N], BF16, tag=f"oT{b}", name=f"oT{b}") for b in range(B)]
        ident = pers.tile([128, 128], BF16, tag="ident")
        nc.sync.dma_start(ident[:], ident_in.ap())
        wqb = pers.tile([128, KS, COLS], BF16, tag="wqb")
        nc.sync.dma_start(wqb[:], wqkv_in.ap().rearrange("(k p) c -> p k c", p=128))
        wob = pers.tile([128, D], BF16, tag="wob")
        nc.sync.dma_start(wob[:], wout_in.ap())
        onesd = pers.tile([128, 1], BF16, tag="onesd")    # 1/D for stats matmuls
        nc.vector.memset(onesd[:], 1.0 / D)
        epsc = pers.tile([128, 1], F32, tag="epsc")
        nc.vector.memset(epsc[:], LN_EPS)
        if not bw_zero:
            bwb = pers.tile([1, COLS], BF16, tag="bwb")
            nc.sync.dma_start(bwb[:], bw_in.ap())

        # ---- LN stats + qkv^T, per batch; stage-pipelined emission ----
        xpool = ctx.enter_context(tc.tile_pool(name="xT", bufs=1))
        tree = ctx.enter_context(tc.tile_pool(name="tree", bufs=2))
        rows = ctx.enter_context(tc.tile_pool(name="rows", bufs=2))
        rbc = ctx.enter_context(tc.tile_pool(name="rbc", bufs=3))
        x2p = ctx.enter_context(tc.tile_pool(name="x2p", bufs=2))
        with tc.tile_pool(name="pstat", bufs=1, space="PSUM") as pstat, \
             tc.tile_pool(name="pqkv", bufs=3, space="PSUM") as pqkv, \
             tc.tile_pool(name="pvt", bufs=2, space="PSUM") as pvt:
            xbs = []
            for b in range(B):
                xb = xpool.tile([128, KS, N], BF16, tag=f"xb{b}", name=f"xb{b}")
                for k in range(KS):
                    nc.sync.dma_start(xb[:, k, :],
                                      xT_in.ap()[b, k * 128:(k + 1) * 128, :])
                nc.vector.memset(
                    vA[b][:, :, 64::65].rearrange("p t o -> p (t o)"), 1.0)
                xbs.append(xb)
            dsts = (qT, kT, vT)

            def stage_a(b, nt):
                """x^2 square + slab pre-sum trees -> s12[:,0]=sum x, [:,1]=sum x^2."""
                sl = slice(nt * 512, (nt + 1) * 512)
                xb = xbs[b]
                x2t = x2p.tile([128, KS, 512], BF16, tag="x2", name=f"x2_{b}_{nt}")
                nc.scalar.activation(x2t[:], xb[:, :, sl], AF.Square)
                s12 = tree.tile([128, 2, 512], BF16, tag="s12", name=f"s12_{b}_{nt}")
                a1 = tree.tile([128, 4, 512], BF16, tag="a1")
                c1 = tree.tile([128, 2, 512], BF16, tag="c1")
                nc.vector.tensor_tensor(a1[:], xb[:, 0::2, sl], xb[:, 1::2, sl],
                                        op=AL.add)
                nc.vector.tensor_tensor(c1[:], a1[:, 0::2, :], a1[:, 1::2, :],
                                        op=AL.add)
                nc.gpsimd.tensor_tensor(s12[:, 0, :], c1[:, 0, :], c1[:, 1, :],
                                        op=AL.add)
                a2 = tree.tile([128, 4, 512], BF16, tag="a2")
                c2 = tree.tile([128, 2, 512], BF16, tag="c2")
                nc.vector.tensor_tensor(a2[:], x2t[:, 0::2, :], x2t[:, 1::2, :],
                                        op=AL.add)
                nc.vector.tensor_tensor(c2[:], a2[:, 0::2, :], a2[:, 1::2, :],
                                        op=AL.add)
                nc.gpsimd.tensor_tensor(s12[:, 1, :], c2[:, 0, :], c2[:, 1, :],
                                        op=AL.add)
                return s12

            def stage_b(b, nt, s12):
                """partition-reduce stats, row chain -> broadcast rsig (f32)."""
                mu_ps = pstat.tile([1, 512], F32, tag="mu_ps")
                nc.tensor.matmul(mu_ps[:], onesd[:], s12[:, 0, :],
                                 start=True, stop=True, **MM)
                x2_ps = pstat.tile([1, 512], F32, tag="x2_ps")
                nc.tensor.matmul(x2_ps[:], onesd[:], s12[:, 1, :],
                                 start=True, stop=True, **MM)
                mu2_r = rows.tile([1, 512], F32, tag="mu2_r", bufs=1)
                nc.scalar.activation(mu2_r[:], mu_ps[:], AF.Square)
                var_r = rows.tile([1, 512], F32, tag="var_r", bufs=1)
                nc.vector.tensor_tensor(var_r[:], x2_ps[:], mu2_r[:],
                                        op=AL.subtract)
                sd_r = rows.tile([1, 512], F32, tag="sd_r")
                nc.scalar.activation(sd_r[:], var_r[:], AF.Sqrt, bias=epsc[0:1, :])
                rsig_r = rows.tile([1, 512], F32, tag="rsig_r")
                nc.vector.reciprocal_approx_fast(rsig_r[:], sd_r[:])
                rsig_bc = rbc.tile([128, 512], F32, tag="rsbc")
                nc.gpsimd.partition_broadcast(rsig_bc[:], rsig_r[:], channels=128)
                if debug:
                    sl = slice(nt * 512, (nt + 1) * 512)
                    nc.sync.dma_start(drs_out.ap()[b, :, sl], rsig_bc[:])
                sd_bf = None
                if not bw_zero:
                    sd_bf = rows.tile([1, 512], BF16, tag="sd_bf")
                    nc.vector.tensor_copy(sd_bf[:], sd_r[:])
                return rsig_bc, sd_bf

            def stage_c(b, nt, rsig_bc, sd_bf):
                """qkv matmuls + rsig eviction to qT/kT/vT."""
                sl = slice(nt * 512, (nt + 1) * 512)
                xb = xbs[b]
                for blk in range(3):
                    csl = slice(blk * 128, (blk + 1) * 128)
                    ps = pqkv.tile([128, 512], F32, tag="psqkv")
                    for k in range(KS):
                        nc.tensor.matmul(ps[:], wqb[:, k, csl], xb[:, k, sl],
                                         start=(k == 0),
                                         stop=(k == KS - 1 and bw_zero), **MM)
                    if not bw_zero:
                        nc.tensor.matmul(ps[:], bwb[0:1, csl], sd_bf[0:1, :],
                                         start=False, stop=True, **MM)
                    nc.vector.tensor_tensor(dsts[blk][b][:, sl], ps[:],
                                            rsig_bc[:], op=AL.mult)

            def stage_d(b, nt):
                """V -> natural layout: 4 PE transposes + one ScalarE copy."""
                pst = pvt.tile([128, 4, 128], BF16, tag="pst")
                for i, tk in enumerate(range(4 * nt, 4 * nt + 4)):
                    nc.tensor.transpose(
                        pst[:, i, :], vT[b][:, tk * 128:(tk + 1) * 128], ident[:])
                nc.scalar.copy(
                    vA[b][:, 4 * nt:4 * nt + 4, :]
                    .rearrange("p t (h v) -> p t h v", h=2)[:, :, :, 0:64],
                    pst[:].rearrange("p t (h v) -> p t h v", h=2))

            # pipelined emission across the 8 (b, nt) blocks:
            # A(i+1) before B(i)/C(i); D(i-1) after C(i)
            blocks = [(b, nt) for b in range(B) for nt in range(IT)]
            s12s = {}
            s12s[blocks[0]] = stage_a(*blocks[0])
            for i, (b, nt) in enumerate(blocks):
                if i + 1 < len(blocks):
                    s12s[blocks[i + 1]] = stage_a(*blocks[i + 1])
                rsig_bc, sd_bf = stage_b(b, nt, s12s.pop((b, nt)))
                stage_c(b, nt, rsig_bc, sd_bf)
                if i > 0:
                    stage_d(*blocks[i - 1])
            stage_d(*blocks[-1])
            if debug:
                for b in range(B):
                    for blk2, srcT in enumerate((qT, kT, vT)):
                        nc.sync.dma_start(dq_out.ap()[b, blk2], srcT[b][:])
                    nc.sync.dma_start(dva_out.ap()[b],
                                      vA[b][:].rearrange("p t o -> p (t o)"))

        # ---- attention + interleaved out-projection ----
        # batch is the outer loop: only one head-pair of P@V accumulators is
        # live (2 banks; a PSUM bank holds exactly one accumulation group),
        # leaving 3 two-bank score slots. P@V matmuls are emitted one
        # (group, head-pair) late so the PE never head-of-line blocks on an
        # exp; the out-projection for i-tile t is emitted inside the next
        # tile's group stream.
        with tc.tile_pool(name="bias", bufs=5) as bias_pool, \
             tc.tile_pool(name="pexp", bufs=3) as exp_pool, \
             tc.tile_pool(name="lnrm", bufs=2) as lnrm, \
             tc.tile_pool(name="ysb", bufs=2) as ysb, \
             tc.tile_pool(name="scp", bufs=3, space="PSUM") as scp, \
             tc.tile_pool(name="psop", bufs=1, space="PSUM") as psop:

            def emit_pv(b, h, pso_h, pe, g, t, nj):
                for si, (j, off) in enumerate(g):
                    pv_off = max(0, 128 * j - 512 * t)
                    nc.tensor.matmul(
                        pso_h[:, pv_off:], vA[b][:, j, h * 65:h * 65 + 65],
                        pe[:, si, pv_off:],
                        start=(j == 0), stop=(j == nj - 1), **MM)

            def emit_proj(b, t):
                for tt in range(4 * t, 4 * t + 4):
                    psy = scp.tile([128, 2, 512], F32, tag="scps", name="psy")
                    for half in range(2):
                        nc.tensor.matmul(psy[:, half, :],
                                         oT[b][:, tt * 128:(tt + 1) * 128],
                                         wob[:, half * 512:(half + 1) * 512],
                                         start=True, stop=True, **MM)
                    yt = ysb.tile([128, D], BF16, tag="yt")
                    nc.vector.tensor_copy(yt[:],
                                          psy[:].rearrange("p a b -> p (a b)"))
                    nc.sync.dma_start(y_out.ap()[b, tt * 128:(tt + 1) * 128, :],
                                      yt[:])

            pend_pv = []      # delayed P@V emissions
            pend_proj = []    # delayed out-projection emissions
            for b in range(B):
                for t in range(IT):
                    isl = slice(t * 512, (t + 1) * 512)
                    nj = 4 * (t + 1)
                    # j-tile groups: pairs over 0..4t+1 (diagonal tile 4t+1
                    # computed full width; host bias -1e9 masks it), then two
                    # trimmed singles
                    groups = [[(2 * k, 0), (2 * k + 1, 0)]
                              for k in range(2 * t + 1)]
                    groups.append([(4 * t + 2, 256)])
                    groups.append([(4 * t + 3, 384)])
                    pso = {h: psop.tile([65, 512], F32, tag=f"pso{h}",
                                        name=f"pso{h}") for h in range(HL)}
                    for g in groups:
                        bts = []
                        for h in range(HL):
                            bt = bias_pool.tile([128, 2, 512], BF16, tag="bt")
                            for si, (j, off) in enumerate(g):
                                nc.sync.dma_start(
                                    bt[:, si, off:],
                                    biasT_in.ap()[h, j * 128:(j + 1) * 128,
                                                  t * 512 + off:(t + 1) * 512])
                            bts.append(bt)
                        scs = [scp.tile([128, 2, 512], F32, tag="scps",
                                        name=f"sc{h}") for h in range(HL)]
                        # scores h0/h1 back-to-back -> concurrent PE row groups
                        for si, (j, off) in enumerate(g):
                            for h in range(HL):
                                hsl = slice(h * 64, (h + 1) * 64)
                                nc.tensor.matmul(
                                    scs[h][:, si, off:],
                                    kT[b][hsl, j * 128:(j + 1) * 128],
                                    qT[b][hsl, t * 512 + off:(t + 1) * 512],
                                    start=True, stop=False, **MM)
                        for si, (j, off) in enumerate(g):
                            for h in range(HL):
                                nc.tensor.matmul(
                                    scs[h][:, si, off:], ident[:],
                                    bts[h][:, si, off:],
                                    start=False, stop=True, **MM)
                        for h in range(HL):
                            pe = exp_pool.tile([128, 2, 512], BF16, tag="pe")
                            if len(g) == 2:
                                nc.scalar.activation(pe[:], scs[h][:], AF.Exp)
                            else:
                                off = g[0][1]
                                nc.scalar.activation(pe[:, 0, off:],
                                                     scs[h][:, 0, off:], AF.Exp)
                            pend_pv.append((b, h, pso[h], pe, g, t, nj))
                            # lag-2 P@V emission (one full head-pair behind)
                            if len(pend_pv) > 2:
                                emit_pv(*pend_pv.pop(0))
                        if pend_proj:
                            emit_proj(*pend_proj.pop(0))
                    while pend_pv:
                        emit_pv(*pend_pv.pop(0))
                    # deferred softmax normalization + eviction to oT
                    for h in range(HL):
                        hsl = slice(h * 64, (h + 1) * 64)
                        lrow = lnrm.tile([1, 512], F32, tag="lrow", bufs=1)
                        nc.vector.tensor_copy(lrow[:], pso[h][64:65, :])
                        rec = lnrm.tile([1, 512], F32, tag="rec")
                        nc.vector.reciprocal_approx_fast(rec[:], lrow[:])
                        if debug:
                            nc.sync.dma_start(drec_out.ap()[b, h, t], rec[:])
                        lb = lnrm.tile([64, 512], F32, tag="lb")
                        nc.gpsimd.partition_broadcast(lb[:], rec[:], channels=64)
                        nc.vector.tensor_tensor(
                            oT[b][hsl, isl], pso[h][0:64, :], lb[:], op=AL.mult)
                    if debug:
                        nc.sync.dma_start(do_out.ap()[b, :, isl], oT[b][:, isl])
                    pend_proj.append((b, t))
            while pend_proj:
                emit_proj(*pend_proj.pop(0))

    nc.compile()
    return nc


_NC_CACHE = {}


def _get_program(bw_zero=True):
    if bw_zero not in _NC_CACHE:
        _NC_CACHE[bw_zero] = build_program(bw_zero)
    return _NC_CACHE[bw_zero]


def build_in_maps(x, attn_bias, ln_gamma, ln_beta, w_qkv, w_out):
    x = np.asarray(x, dtype=np.float32)
    attn_bias = np.asarray(attn_bias, dtype=np.float32)
    ln_gamma = np.asarray(ln_gamma, dtype=np.float32)
    ln_beta = np.asarray(ln_beta, dtype=np.float32)
    w_qkv = np.asarray(w_qkv, dtype=np.float32)
    w_out = np.asarray(w_out, dtype=np.float32)

    ident = np.eye(128, dtype=ml_dtypes.bfloat16)
    xT = np.ascontiguousarray(x.transpose(0, 2, 1)).astype(ml_dtypes.bfloat16)
    # causal mask folded into the bias, transposed to [head, key j, query i]
    tri = np.triu(np.ones((N, N), dtype=bool), k=1)  # True above diag (masked)
    in_maps = []
    for c in range(N_CORES):
        h0 = HL * c
        cols = np.concatenate([
            w_qkv[:, q * H * DH + h0 * DH: q * H * DH + (h0 + HL) * DH]
            for q in range(3)], axis=1)
        # gamma scaling + attention scale on the q block
        cols = cols * ln_gamma[:, None]
        cols[:, 0:128] *= SCALE
        # beta @ W row (before mean-fold; the fold cancels against mu anyway)
        bw = (ln_beta @ cols)[None, :]
        # fold the LN mean subtraction into the weights:
        # x^T (W - 1 colsum(W)/D) = (x - mu)^T W
        cols = cols - cols.sum(axis=0, keepdims=True) / D
        biasT = np.empty((HL, N, N), dtype=ml_dtypes.bfloat16)
        for h in range(HL):
            bh = attn_bias[h0 + h].copy()
            bh[tri] = NEG
            biasT[h] = bh.T.astype(ml_dtypes.bfloat16)
        in_maps.append({
            "xT": xT,
            "biasT": biasT,
            "wqkv": np.ascontiguousarray(cols).astype(ml_dtypes.bfloat16),
            "wout": np.ascontiguousarray(
                w_out[h0 * DH:(h0 + HL) * DH]).astype(ml_dtypes.bfloat16),
            "bw": bw.astype(ml_dtypes.bfloat16),
            "ident": ident,
        })
    return in_maps


def kernel(x, attn_bias, ln_gamma, ln_beta, w_qkv, w_out):
    in_maps = build_in_maps(x, attn_bias, ln_gamma, ln_beta, w_qkv, w_out)
    bw_zero = all(np.all(m["bw"] == 0) for m in in_maps)
    nc = _get_program(bw_zero)
    res = run_bass_kernel_spmd(nc, in_maps, core_ids=list(range(N_CORES)))
    out = np.zeros((B, N, D), dtype=np.float32)
    for c in range(N_CORES):
        out += res.results[c]["y"].astype(np.float32)
    return out


# revision 22
# speedup vs baseline: 1.4190x; 1.1715x over previous
"""Fused LayerNorm + causal multi-head attention (with additive bias) + out-proj
for Trainium2, SPMD over 8 NeuronCores.

Sharding: tensor-parallel over heads. 16 heads / 8 cores = 2 heads per core.
Each core computes LN(x) (replicated), the qkv projection restricted to its
2 heads' columns, causal softmax attention with its heads' bias slices, and a
partial output projection (its heads' rows of w_out). Host sums the 8 partial
outputs (the TP all-reduce, done on gather) in f32 from bf16 partials.

Key choices (engine queues are strict FIFO, so emission order is software-
pipelined to keep the PE fed):
 - x arrives pre-transposed ([dim, token], bf16). The LN mean-subtraction is
   folded into the weights on the HOST: W'' = gW - (1/D) 1 colsum(gW), so
   x^T W'' = (x - mu)^T gW directly. Only rsig (1/std) is applied on device,
   riding the PSUM->SBUF eviction (DVE multiply against a broadcast f32 rsig
   row). The beta term is a rank-1 PSUM matmul (skipped when beta == 0).
 - LN variance stats: slab pre-sums of x and x^2 run on the DVE as strided
   tensor-tensor adds (one instruction does 4 adds), last level on GpSimd;
   the 128-partition reduction is one ones-vector matmul per 512-token block
   (8k PE cycles total vs 65k for direct stat matmuls). x^2 comes from one
   ScalarE Square over [128, 8, 512] per block. Stage emission is pipelined:
   squares/trees for block nt+1 are emitted before block nt's consumers, and
   the V transposes for nt-1 after block nt's qkv matmuls, so no engine FIFO
   head-of-line blocks on a cross-engine chain.
 - Scores are computed transposed, S^T[j, i] = (k_j . q_i); the two heads'
   C=64 matmuls are emitted back-to-back and auto-pair into PE row groups
   (0,0)/(64,0), running concurrently.
 - The causal mask and softmax max-subtraction are folded into the host bias
   (pre-masked with -1e9; logits are O(10) so exp never overflows). The
   diagonal-adjacent j-tile is computed full width so it pairs with its
   neighbour in one exp; only fully-masked regions of the two outer diagonal
   tiles are trimmed.
 - Score PSUM tiles span two banks ([128, 2, 512]) so one exp ACTIVATE covers
   two j-tiles (~293ns fixed cost per ACTIVATE). Batch is the OUTER attention
   loop so only one head-pair of P@V accumulators is live (2 banks; a PSUM
   bank supports exactly one accumulation group), leaving 3 score slots; P@V
   matmuls are emitted one (group, head-pair) late so the PE never stalls on
   an exp.
 - Softmax normalization is deferred: an all-ones column appended to V gives
   the row sums l_i for free; 1/l is applied to O^T after P@V.
 - The out-projection for each 512-token i-tile is emitted one loop step
   late, interleaving with the next tile's attention; y is written bf16.
"""

import numpy as np
import ml_dtypes
from contextlib import ExitStack

import concourse.bass as bass
import concourse.tile as tile
from concourse import bacc, mybir
from concourse.bass_utils import run_bass_kernel_spmd

F32 = mybir.dt.float32
BF16 = mybir.dt.bfloat16
AL = mybir.AluOpType
AF = mybir.ActivationFunctionType

N_CORES = 8
B = 2            # batch
N = 2048         # tokens
D = 1024         # model dim
H = 16           # total heads
HL = 2           # heads per core
DH = 64          # head dim
COLS = 3 * HL * DH   # 384 qkv columns per core
KS = D // 128    # 8 contraction slabs
TT = N // 128    # 16 token tiles
IT = N // 512    # 4 i-tiles (query tiles of 512)
SCALE = DH ** -0.5
LN_EPS = 1e-5
NEG = -1.0e9


def build_program(bw_zero=True, debug=False):
    nc = bacc.Bacc("TRN2", target_bir_lowering=False, debug=False)

    xT_in = nc.dram_tensor("xT", [B, D, N], BF16, kind="ExternalInput")
    biasT_in = nc.dram_tensor("biasT", [HL, N, N], BF16, kind="ExternalInput")
    wqkv_in = nc.dram_tensor("wqkv", [D, COLS], BF16, kind="ExternalInput")
    wout_in = nc.dram_tensor("wout", [HL * DH, D], BF16, kind="ExternalInput")
    bw_in = nc.dram_tensor("bw", [1, COLS], BF16, kind="ExternalInput")
    ident_in = nc.dram_tensor("ident", [128, 128], BF16, kind="ExternalInput")
    y_out = nc.dram_tensor("y", [B, N, D], BF16, kind="ExternalOutput")
    if debug:
        dq_out = nc.dram_tensor("dq", [B, 3, 128, N], BF16, kind="ExternalOutput")
        drs_out = nc.dram_tensor("drs", [B, 128, N], F32, kind="ExternalOutput")
        do_out = nc.dram_tensor("do", [B, 128, N], BF16, kind="ExternalOutput")
        dva_out = nc.dram_tensor("dva", [B, 128, TT * 130], BF16, kind="ExternalOutput")
        drec_out = nc.dram_tensor("drec", [B, HL, IT, 512], F32, kind="ExternalOutput")

    MM = dict(skip_group_check=True)

    with tile.TileContext(nc) as tc, ExitStack() as ctx:
        # ---- persistent sbuf ----
        pers = ctx.enter_context(tc.tile_pool(name="pers", bufs=1))
        qT = [pers.tile([128, N], BF16, tag=f"qT{b}", name=f"qT{b}") for b in range(B)]
        kT = [pers.tile([128, N], BF16, tag=f"kT{b}", name=f"kT{b}") for b in range(B)]
        vT = [pers.tile([128, N], BF16, tag=f"vT{b}", name=f"vT{b}") for b in range(B)]
        # V natural with ones column: per key-tile [.., 130]: h0 v(64)+1, h1 v(64)+1
        vA = [pers.tile([128, TT, 130], BF16, tag=f"vA{b}", name=f"vA{b}") for b in range(B)]
        oT = [pers.tile([128, N], BF16, tag=f"oT{b}", name=f"oT{b}") for b in range(B)]
        ident = pers.tile([128, 128], BF16, tag="ident")
        nc.sync.dma_start(ident[:], ident_in.ap())
        wqb = pers.tile([128, KS, COLS], BF16, tag="wqb")
        nc.sync.dma_start(wqb[:], wqkv_in.ap().rearrange("(k p) c -> p k c", p=128))
        wob = pers.tile([128, D], BF16, tag="wob")
        nc.sync.dma_start(wob[:], wout_in.ap())
        onesd = pers.tile([128, 1], BF16, tag="onesd")    # 1/D for stats matmuls
        nc.vector.memset(onesd[:], 1.0 / D)
        epsc = pers.tile([128, 1], F32, tag="epsc")
        nc.vector.memset(epsc[:], LN_EPS)
        if not bw_zero:
            bwb = pers.tile([1, COLS], BF16, tag="bwb")
            nc.sync.dma_start(bwb[:], bw_in.ap())

        # ---- LN stats + qkv^T, per batch; stage-pipelined emission ----
        xpool = ctx.enter_context(tc.tile_pool(name="xT", bufs=1))
        tree = ctx.enter_context(tc.tile_pool(name="tree", bufs=2))
        rows = ctx.enter_context(tc.tile_pool(name="rows", bufs=2))
        rbc = ctx.enter_context(tc.tile_pool(name="rbc", bufs=3))
        x2p = ctx.enter_context(tc.tile_pool(name="x2p", bufs=2))
        with tc.tile_pool(name="pstat", bufs=1, space="PSUM") as pstat, \
             tc.tile_pool(name="pqkv", bufs=3, space="PSUM") as pqkv, \
             tc.tile_pool(name="pvt", bufs=2, space="PSUM") as pvt:
            xbs = []
            for b in range(B):
                xb = xpool.tile([128, KS, N], BF16, tag=f"xb{b}", name=f"xb{b}")
                for k in range(KS):
                    nc.sync.dma_start(xb[:, k, :],
                                      xT_in.ap()[b, k * 128:(k + 1) * 128, :])
                nc.vector.memset(
                    vA[b][:, :, 64::65].rearrange("p t o -> p (t o)"), 1.0)
                xbs.append(xb)
            dsts = (qT, kT, vT)

            def stage_a(b, nt):
                """x^2 square (ScalarE) + x slab pre-sum tree (DVE only)."""
                sl = slice(nt * 512, (nt + 1) * 512)
                xb = xbs[b]
                x2t = x2p.tile([128, KS, 512], BF16, tag="x2", name=f"x2_{b}_{nt}")
                nc.scalar.activation(x2t[:], xb[:, :, sl], AF.Square)
                a1 = tree.tile([128, 4, 512], BF16, tag="a1", name=f"a1_{b}_{nt}")
                c1 = tree.tile([128, 2, 512], BF16, tag="c1", name=f"c1_{b}_{nt}")
                nc.vector.tensor_tensor(a1[:], xb[:, 0::2, sl], xb[:, 1::2, sl],
                                        op=AL.add)
                nc.vector.tensor_tensor(c1[:], a1[:, 0::2, :], a1[:, 1::2, :],
                                        op=AL.add)
                return (x2t, c1)

            def stage_b(b, nt, x2t, c1):
                """stat matmuls (PE-internal accumulation), row chain, rsig."""
                mu_ps = pstat.tile([1, 512], F32, tag="mu_ps")
                for i in range(2):
                    nc.tensor.matmul(mu_ps[:], onesd[:], c1[:, i, :],
                                     start=(i == 0), stop=(i == 1), **MM)
                x2_ps = pstat.tile([1, 512], F32, tag="x2_ps")
                for k in range(KS):
                    nc.tensor.matmul(x2_ps[:], onesd[:], x2t[:, k, :],
                                     start=(k == 0), stop=(k == KS - 1), **MM)
                mu2_r = rows.tile([1, 512], F32, tag="mu2_r", bufs=1)
                nc.scalar.activation(mu2_r[:], mu_ps[:], AF.Square)
                var_r = rows.tile([1, 512], F32, tag="var_r", bufs=1)
                nc.vector.tensor_tensor(var_r[:], x2_ps[:], mu2_r[:],
                                        op=AL.subtract)
                sd_r = rows.tile([1, 512], F32, tag="sd_r")
                nc.scalar.activation(sd_r[:], var_r[:], AF.Sqrt, bias=epsc[0:1, :])
                rsig_r = rows.tile([1, 512], F32, tag="rsig_r")
                nc.vector.reciprocal_approx_fast(rsig_r[:], sd_r[:])
                rsig_bc = rbc.tile([128, 512], F32, tag="rsbc")
                nc.gpsimd.partition_broadcast(rsig_bc[:], rsig_r[:], channels=128)
                if debug:
                    sl = slice(nt * 512, (nt + 1) * 512)
                    nc.sync.dma_start(drs_out.ap()[b, :, sl], rsig_bc[:])
                sd_bf = None
                if not bw_zero:
                    sd_bf = rows.tile([1, 512], BF16, tag="sd_bf")
                    nc.vector.tensor_copy(sd_bf[:], sd_r[:])
                return rsig_bc, sd_bf

            def stage_c(b, nt, rsig_bc, sd_bf):
                """qkv matmuls + rsig eviction to qT/kT/vT."""
                sl = slice(nt * 512, (nt + 1) * 512)
                xb = xbs[b]
                for blk in range(3):
                    csl = slice(blk * 128, (blk + 1) * 128)
                    ps = pqkv.tile([128, 512], F32, tag="psqkv")
                    for k in range(KS):
                        nc.tensor.matmul(ps[:], wqb[:, k, csl], xb[:, k, sl],
                                         start=(k == 0),
                                         stop=(k == KS - 1 and bw_zero), **MM)
                    if not bw_zero:
                        nc.tensor.matmul(ps[:], bwb[0:1, csl], sd_bf[0:1, :],
                                         start=False, stop=True, **MM)
                    nc.vector.tensor_tensor(dsts[blk][b][:, sl], ps[:],
                                            rsig_bc[:], op=AL.mult)

            def stage_d(b, nt):
                """V -> natural layout: 4 PE transposes + one ScalarE copy."""
                pst = pvt.tile([128, 4, 128], BF16, tag="pst")
                for i, tk in enumerate(range(4 * nt, 4 * nt + 4)):
                    nc.tensor.transpose(
                        pst[:, i, :], vT[b][:, tk * 128:(tk + 1) * 128], ident[:])
                nc.scalar.copy(
                    vA[b][:, 4 * nt:4 * nt + 4, :]
                    .rearrange("p t (h v) -> p t h v", h=2)[:, :, :, 0:64],
                    pst[:].rearrange("p t (h v) -> p t h v", h=2))

            # pipelined emission across the 8 (b, nt) blocks:
            # A(i+1) before B(i)/C(i); D(i-1) after C(i)
            blocks = [(b, nt) for b in range(B) for nt in range(IT)]
            pre = {}
            pre[blocks[0]] = stage_a(*blocks[0])
            for i, (b, nt) in enumerate(blocks):
                if i + 1 < len(blocks):
                    pre[blocks[i + 1]] = stage_a(*blocks[i + 1])
                x2t, c1 = pre.pop((b, nt))
                rsig_bc, sd_bf = stage_b(b, nt, x2t, c1)
                stage_c(b, nt, rsig_bc, sd_bf)
                if i > 0:
                    stage_d(*blocks[i - 1])
            stage_d(*blocks[-1])
            if debug:
                for b in range(B):
                    for blk2, srcT in enumerate((qT, kT, vT)):
                        nc.sync.dma_start(dq_out.ap()[b, blk2], srcT[b][:])
                    nc.sync.dma_start(dva_out.ap()[b],
                                      vA[b][:].rearrange("p t o -> p (t o)"))

        # ---- attention + interleaved out-projection ----
        # Four single-bank score slots + per-(b,h) P@V accumulators (4 banks).
        # Each j-step spawns 4 (b, h) units: score -> bias-ident -> exp -> P@V,
        # with P@V emission lagged 4 units and the out-projection lagged one
        # i-tile, so the strict-FIFO engine queues never head-of-line block.
        with tc.tile_pool(name="bias", bufs=8) as bias_pool, \
             tc.tile_pool(name="pexp", bufs=6) as exp_pool, \
             tc.tile_pool(name="lnrm", bufs=2) as lnrm, \
             tc.tile_pool(name="ysb", bufs=3) as ysb, \
             tc.tile_pool(name="scp", bufs=4, space="PSUM") as scp, \
             tc.tile_pool(name="psop", bufs=1, space="PSUM") as psop:

            def emit_pv(b, h, pso_bh, pe, j, off, t, nj):
                nc.tensor.matmul(
                    pso_bh[:, off:], vA[b][:, j, h * 65:h * 65 + 65],
                    pe[:, off:], start=(j == 0), stop=(j == nj - 1), **MM)

            def emit_proj(b, t):
                for tt in range(4 * t, 4 * t + 4):
                    for half in range(2):
                        psy = scp.tile([128, 512], F32, tag="scps", name="psy")
                        nc.tensor.matmul(psy[:],
                                         oT[b][:, tt * 128:(tt + 1) * 128],
                                         wob[:, half * 512:(half + 1) * 512],
                                         start=True, stop=True, **MM)
                        yt = ysb.tile([128, 512], BF16, tag="yt")
                        nc.vector.tensor_copy(yt[:], psy[:])
                        nc.sync.dma_start(
                            y_out.ap()[b, tt * 128:(tt + 1) * 128,
                                       half * 512:(half + 1) * 512], yt[:])

            pend_pv = []      # delayed P@V emissions
            pend_proj = []    # delayed out-projection emissions
            for t in range(IT):
                isl = slice(t * 512, (t + 1) * 512)
                nj = 4 * (t + 1)
                pso = {(b, h): psop.tile([65, 512], F32, tag=f"pso{b}{h}",
                                         name=f"pso{b}{h}")
                       for b in range(B) for h in range(HL)}
                for j in range(nj):
                    off = max(0, 128 * j - 512 * t)
                    jsl = slice(j * 128, (j + 1) * 128)
                    islo = slice(t * 512 + off, (t + 1) * 512)
                    bts = []
                    for h in range(HL):
                        bt = bias_pool.tile([128, 512], BF16, tag="bt")
                        nc.sync.dma_start(bt[:, off:], biasT_in.ap()[h, jsl, islo])
                        bts.append(bt)
                    for b in range(B):
                        scs = [scp.tile([128, 512], F32, tag="scps",
                                        name=f"sc{h}") for h in range(HL)]
                        # h0/h1 back-to-back -> concurrent PE row groups
                        for h in range(HL):
                            hsl = slice(h * 64, (h + 1) * 64)
                            nc.tensor.matmul(scs[h][:, off:], kT[b][hsl, jsl],
                                             qT[b][hsl, islo],
                                             start=True, stop=False, **MM)
                        for h in range(HL):
                            nc.tensor.matmul(scs[h][:, off:], ident[:],
                                             bts[h][:, off:],
                                             start=False, stop=True, **MM)
                        for h in range(HL):
                            pe = exp_pool.tile([128, 512], BF16, tag="pe")
                            nc.scalar.activation(pe[:, off:], scs[h][:, off:],
                                                 AF.Exp)
                            pend_pv.append((b, h, pso[(b, h)], pe, j, off, t, nj))
                            if len(pend_pv) > 4:
                                emit_pv(*pend_pv.pop(0))
                    if j < 2 and pend_proj:
                        emit_proj(*pend_proj.pop(0))
                while pend_pv:
                    emit_pv(*pend_pv.pop(0))
                # deferred softmax normalization + eviction to oT
                for b in range(B):
                    for h in range(HL):
                        hsl = slice(h * 64, (h + 1) * 64)
                        lrow = lnrm.tile([1, 512], F32, tag="lrow", bufs=1)
                        nc.vector.tensor_copy(lrow[:], pso[(b, h)][64:65, :])
                        rec = lnrm.tile([1, 512], F32, tag="rec")
                        nc.vector.reciprocal_approx_fast(rec[:], lrow[:])
                        if debug:
                            nc.sync.dma_start(drec_out.ap()[b, h, t], rec[:])
                        lb = lnrm.tile([64, 512], F32, tag="lb")
                        nc.gpsimd.partition_broadcast(lb[:], rec[:], channels=64)
                        nc.vector.tensor_tensor(
                            oT[b][hsl, isl], pso[(b, h)][0:64, :], lb[:],
                            op=AL.mult)
                if debug:
                    for b in range(B):
                        nc.sync.dma_start(do_out.ap()[b, :, isl], oT[b][:, isl])
                for b in range(B):
                    pend_proj.append((b, t))
            while pend_proj:
                emit_proj(*pend_proj.pop(0))

    nc.compile()
    return nc


_NC_CACHE = {}


def _get_program(bw_zero=True):
    if bw_zero not in _NC_CACHE:
        _NC_CACHE[bw_zero] = build_program(bw_zero)
    return _NC_CACHE[bw_zero]


def build_in_maps(x, attn_bias, ln_gamma, ln_beta, w_qkv, w_out):
    x = np.asarray(x, dtype=np.float32)
    attn_bias = np.asarray(attn_bias, dtype=np.float32)
    ln_gamma = np.asarray(ln_gamma, dtype=np.float32)
    ln_beta = np.asarray(ln_beta, dtype=np.float32)
    w_qkv = np.asarray(w_qkv, dtype=np.float32)
    w_out = np.asarray(w_out, dtype=np.float32)

    ident = np.eye(128, dtype=ml_dtypes.bfloat16)
    xT = np.ascontiguousarray(x.transpose(0, 2, 1)).astype(ml_dtypes.bfloat16)
    # causal mask folded into the bias, transposed to [head, key j, query i]
    tri = np.triu(np.ones((N, N), dtype=bool), k=1)  # True above diag (masked)
    in_maps = []
    for c in range(N_CORES):
        h0 = HL * c
        cols = np.concatenate([
            w_qkv[:, q * H * DH + h0 * DH: q * H * DH + (h0 + HL) * DH]
            for q in range(3)], axis=1)
        # gamma scaling + attention scale on the q block
        cols = cols * ln_gamma[:, None]
        cols[:, 0:128] *= SCALE
        # beta @ W row (before mean-fold; the fold cancels against mu anyway)
        bw = (ln_beta @ cols)[None, :]
        # fold the LN mean subtraction into the weights:
        # x^T (W - 1 colsum(W)/D) = (x - mu)^T W
        cols = cols - cols.sum(axis=0, keepdims=True) / D
        biasT = np.empty((HL, N, N), dtype=ml_dtypes.bfloat16)
        for h in range(HL):
            bh = attn_bias[h0 + h].copy()
            bh[tri] = NEG
            biasT[h] = bh.T.astype(ml_dtypes.bfloat16)
        in_maps.append({
            "xT": xT,
            "biasT": biasT,
            "wqkv": np.ascontiguousarray(cols).astype(ml_dtypes.bfloat16),
            "wout": np.ascontiguousarray(
                w_out[h0 * DH:(h0 + HL) * DH]).astype(ml_dtypes.bfloat16),
            "bw": bw.astype(ml_dtypes.bfloat16),
            "ident": ident,
        })
    return in_maps


def kernel(x, attn_bias, ln_gamma, ln_beta, w_qkv, w_out):
    in_maps = build_in_maps(x, attn_bias, ln_gamma, ln_beta, w_qkv, w_out)
    bw_zero = all(np.all(m["bw"] == 0) for m in in_maps)
    nc = _get_program(bw_zero)
    res = run_bass_kernel_spmd(nc, in_maps, core_ids=list(range(N_CORES)))
    out = np.zeros((B, N, D), dtype=np.float32)
    for c in range(N_CORES):
        out += res.results[c]["y"].astype(np.float32)
    return out
